# revision 10
# baseline (speedup 1.0000x reference)
"""Bass/Tile TRN2 kernel for nn_DecoderRNN (attention + 4-layer GRU decoder + BN scoring).

Sharding: data-parallel over batch B=64 across 8 NeuronCores (8 batches/core).
Layout: feature-on-partition, batch-on-free ("transposed") throughout.
All recurrent-loop matmuls in fp32 (the recurrence is chaotic: sharp softmax
argmax flips amplify any sub-fp32 noise ~3000x; bf16/fp16 fail 2e-2).
SBUF cannot hold fp32 weights + keys/values, so cell1-input/cell2/cell3
weights (~15MB) stream from HBM every step, hidden under PE time.
Scoring batchnorm uses 2 tiny AllReduces for global (16000-row) statistics.
"""
import sys

sys.path.insert(0, '/opt/trn_rl_repo')

import numpy as np

import concourse.bass as bass
import concourse.mybir as mybir
import concourse.tile as tile
from concourse import bacc
from concourse import bass_utils
from concourse.bass import ds

F32 = mybir.dt.float32

NUM_CHARS = 34
KEY = 128
VAL = 256
HID = 512
T_ENC = 1024
B = 64
L = 250
N_CORES = 8
BL = B // N_CORES          # local batch = 8
NT = T_ENC // 128          # 8 t-chunks
BN_EPS = 1e-5
NROWS = B * L              # 16000 global BN rows

_cached_nc = None
DEBUG_H3 = False
DEBUG_STAGE = False


def _mm(nc, out, lhsT, rhs, start, stop):
    nc.tensor.matmul(out, lhsT, rhs, start=start, stop=stop, skip_group_check=True)


def _build():
    nc = bacc.Bacc('TRN2', target_bir_lowering=False, debug=False,
                   num_devices=N_CORES)

    def din(name, shape):
        return nc.dram_tensor(name, list(shape), F32, kind='ExternalInput')

    # --- DRAM inputs (host-packed into SBUF-exact layouts) ---
    KTd = din('KTp', [128, BL * NT * 128])            # [k, (b,tc,t)]
    VTd = din('VTp', [128, BL * NT * 2 * 128])        # [t, (b,tc,vc,v)]
    maskd = din('maskp', [128, NT * BL])              # [t, (tc,b)]
    pre0d = din('pre0p', [L * 128, 18 * BL])          # row t*128+r, col mc*8+b
    wqTd = din('wqTp', [128, 4 * 128])                # (kc)
    W0d = din('W0p', [128, 2 * 18 * 128])             # (kc*18+mc)  resident
    W1rzd = din('W1rzp', [128, 10 * 8 * 128])         # (kc*8+mc)   resident
    # streamed weights: MC-MAJOR (mc*nk+kc) so one stage tile = one psum group
    W1hnd = din('W1hnp', [128, 4 * 4 * 128])          # (mc*4+kc)   streamed
    W1ind = din('W1inp', [128, 4 * 6 * 128])          # (mc*6+kc)   streamed
    W2rzd = din('W2rzp', [128, 8 * 8 * 128])          # (mc*8+kc)   streamed
    W2ind = din('W2inp', [128, 4 * 4 * 128])          # (mc*4+kc)   streamed
    W2hnd = din('W2hnp', [128, 4 * 4 * 128])          # streamed
    W3rzd = din('W3rzp', [128, 8 * 8 * 128])          # streamed
    W3ind = din('W3inp', [128, 4 * 4 * 128])          # streamed
    W3hnd = din('W3hnp', [128, 4 * 4 * 128])          # streamed
    w1Td = din('w1Tp', [128, 4 * 4 * 128])            # (kc*4+mc)
    w2Td = din('w2Tp', [128, 4 * 34])                 # (kc)
    bqd = din('bqp', [128, 1])
    hn0d = din('hn0p', [128, 6])
    brz1d = din('brz1p', [128, 8])
    bin1d = din('bin1p', [128, 4])
    bhn1d = din('bhn1p', [128, 4])
    brz2d = din('brz2p', [128, 8])
    bin2d = din('bin2p', [128, 4])
    bhn2d = din('bhn2p', [128, 4])
    brz3d = din('brz3p', [128, 8])
    bin3d = din('bin3p', [128, 4])
    bhn3d = din('bhn3p', [128, 4])
    h10d = din('h10p', [128, 32])
    h20d = din('h20p', [128, 32])
    h30d = din('h30p', [128, 32])
    fhTd = din('fhTp', [128, 32])
    g1d = din('g1p', [128, 4])
    be1d = din('be1p', [128, 4])
    bb1d = din('bb1p', [128, 4])
    g2d = din('g2p', [128, 4])
    be2d = din('be2p', [128, 4])
    bb2d = din('bb2p', [34, 1])
    outd = nc.dram_tensor('outp', [34, L * BL], F32, kind='ExternalOutput')
    dbgd = [nc.dram_tensor(f'dbg{k}', [128, L * BL], F32, kind='ExternalOutput')
            for k in range(4)] if DEBUG_H3 else None
    if DEBUG_STAGE:
        stgd = {nm: nc.dram_tensor(f'st_{nm}', [128, w], F32, kind='ExternalOutput')
                for nm, w in [('qf', 8), ('attb', 64), ('invb', 8), ('hv', 48),
                              ('h0f', 48), ('h1f', 32), ('h2f', 32)]}

    SIG = mybir.ActivationFunctionType.Sigmoid
    EXP = mybir.ActivationFunctionType.Exp
    IDN = mybir.ActivationFunctionType.Identity
    SQRT = mybir.ActivationFunctionType.Sqrt
    NL = L * BL            # 2000
    HF = NL // 2           # 1000
    PS = HF // 2           # 500 (scoring matmul piece)

    with tile.TileContext(nc) as tc:
        with tc.tile_pool(name='wpool', bufs=1) as wp, \
             tc.tile_pool(name='hbm', bufs=1, space='DRAM') as dp:
            wqT = wp.tile([128, 4 * 128], F32, tag='wqT')
            W0 = wp.tile([128, 2 * 18 * 128], F32, tag='W0')
            W1rz = wp.tile([128, 10 * 8 * 128], F32, tag='W1rz')
            w1T = wp.tile([128, 4 * 4 * 128], F32, tag='w1T')
            w2T = wp.tile([128, 4 * 34], F32, tag='w2T')
            bq = wp.tile([128, 1], F32, tag='bq')
            hn0c = wp.tile([128, 6], F32, tag='hn0c')
            brz1 = wp.tile([128, 8], F32, tag='brz1')
            bin1 = wp.tile([128, 4], F32, tag='bin1')
            bhn1 = wp.tile([128, 4], F32, tag='bhn1')
            brz2 = wp.tile([128, 8], F32, tag='brz2')
            bin2 = wp.tile([128, 4], F32, tag='bin2')
            bhn2 = wp.tile([128, 4], F32, tag='bhn2')
            brz3 = wp.tile([128, 8], F32, tag='brz3')
            bin3 = wp.tile([128, 4], F32, tag='bin3')
            bhn3 = wp.tile([128, 4], F32, tag='bhn3')
            g1 = wp.tile([128, 4], F32, tag='g1')
            be1 = wp.tile([128, 4], F32, tag='be1')
            bb1 = wp.tile([128, 4], F32, tag='bb1')
            g2 = wp.tile([128, 4], F32, tag='g2')
            be2 = wp.tile([128, 4], F32, tag='be2')
            bb2 = wp.tile([34, 1], F32, tag='bb2')
            h1f = wp.tile([128, 32], F32, tag='h1f')
            h2f = wp.tile([128, 32], F32, tag='h2f')
            h3f = wp.tile([128, 32], F32, tag='h3f')
            hv = wp.tile([128, 48], F32, tag='hv')      # [ctx(16) | fh(32)]
            ones_c = wp.tile([128, 1], F32, tag='ones_c')
            ones_r = wp.tile([1, 128], F32, tag='ones_r')
            # HBM scratch for h3 history (read back for scoring)
            hs = [dp.tile([128, NL], F32, tag=f'hs{k}', name=f'hs{k}') for k in range(4)]

            for tl, dr in [(wqT, wqTd), (W0, W0d), (W1rz, W1rzd),
                           (w1T, w1Td), (w2T, w2Td), (bq, bqd), (hn0c, hn0d),
                           (brz1, brz1d), (bin1, bin1d), (bhn1, bhn1d),
                           (brz2, brz2d), (bin2, bin2d), (bhn2, bhn2d),
                           (brz3, brz3d), (bin3, bin3d), (bhn3, bhn3d),
                           (g1, g1d), (be1, be1d), (bb1, bb1d),
                           (g2, g2d), (be2, be2d), (bb2, bb2d),
                           (h1f, h10d), (h2f, h20d), (h3f, h30d)]:
                nc.sync.dma_start(tl[:], dr.ap())
            nc.sync.dma_start(hv[:, 16:48], fhTd.ap())
            nc.vector.memset(ones_c[:], 1.0)
            nc.vector.memset(ones_r[:], 1.0)

            with tc.tile_pool(name='attn', bufs=1) as ap_, \
                 tc.tile_pool(name='work', bufs=1) as wk, \
                 tc.tile_pool(name='prep', bufs=2) as prep, \
                 tc.tile_pool(name='wstg', bufs=3) as wstg, \
                 tc.tile_pool(name='psA', bufs=4, space='PSUM') as psA, \
                 tc.tile_pool(name='psE', bufs=2, space='PSUM') as psE, \
                 tc.tile_pool(name='psC', bufs=2, space='PSUM') as psC:

                KT = ap_.tile([128, BL * NT * 128], F32, tag='KT')
                VT = ap_.tile([128, BL * NT * 2 * 128], F32, tag='VT')
                maskp = ap_.tile([128, NT * BL], F32, tag='maskp')
                nc.sync.dma_start(KT[:], KTd.ap())
                nc.sync.dma_start(VT[:], VTd.ap())
                nc.sync.dma_start(maskp[:], maskd.ap())

                qf = wk.tile([128, 8], F32, tag='qf')
                attb = wk.tile([128, NT * 8], F32, tag='attb')
                invd = wk.tile([1, 8], F32, tag='invd')
                invb = wk.tile([128, 8], F32, tag='invb')
                rz0p = wk.tile([128, 96], F32, tag='rz0p')
                rz0 = wk.tile([128, 96], F32, tag='rz0')
                hn0t = wk.tile([128, 48], F32, tag='hn0t')
                n0s = wk.tile([128, 48], F32, tag='n0s')
                n0 = wk.tile([128, 48], F32, tag='n0')
                t0 = wk.tile([128, 48], F32, tag='t0')
                h0f = wk.tile([128, 48], F32, tag='h0f')
                rz1 = wk.tile([128, 64], F32, tag='rz1')
                rz2 = wk.tile([128, 64], F32, tag='rz2')
                rz3 = wk.tile([128, 64], F32, tag='rz3')
                hnt = wk.tile([128, 32], F32, tag='hnt')
                int_ = wk.tile([128, 32], F32, tag='int_')
                nts = wk.tile([128, 32], F32, tag='nts')
                ntt = wk.tile([128, 32], F32, tag='ntt')
                tta = wk.tile([128, 32], F32, tag='tta')

                def cell_fused(rzW, rz_nk, rhs_fn, brz, rzt):
                    """rz gates from a RESIDENT kc-major weight tile."""
                    for mc in range(8):
                        ps = psA.tile([128, 8], F32, tag='gps')
                        for kc in range(rz_nk):
                            c = (kc * 8 + mc) * 128
                            _mm(nc, ps[:], rzW[:, c:c + 128], rhs_fn(kc),
                                kc == 0, kc == rz_nk - 1)
                        nc.scalar.activation(rzt[:, mc * 8:(mc + 1) * 8], ps[:], SIG,
                                             bias=brz[:, mc:mc + 1])

                def cell_fused_stream(rzWd, rz_nk, rhs_fn, brz, rzt):
                    """rz gates from a STREAMED mc-major weight (DRAM)."""
                    for mc in range(8):
                        stg = wstg.tile([128, 1024], F32, tag='stg')
                        nc.sync.dma_start(stg[:, 0:rz_nk * 128],
                                          rzWd.ap()[:, mc * rz_nk * 128:(mc + 1) * rz_nk * 128])
                        ps = psA.tile([128, 8], F32, tag='gps')
                        for kc in range(rz_nk):
                            _mm(nc, ps[:], stg[:, kc * 128:(kc + 1) * 128], rhs_fn(kc),
                                kc == 0, kc == rz_nk - 1)
                        nc.scalar.activation(rzt[:, mc * 8:(mc + 1) * 8], ps[:], SIG,
                                             bias=brz[:, mc:mc + 1])

                def quad_stream(Wd, nk, rhs_fn, bias, dst):
                    """4 output chunks (hn or in part) from streamed weights."""
                    for mc in range(4):
                        stg = wstg.tile([128, 1024], F32, tag='stg')
                        nc.sync.dma_start(stg[:, 0:nk * 128],
                                          Wd.ap()[:, mc * nk * 128:(mc + 1) * nk * 128])
                        ps = psA.tile([128, 8], F32, tag='gps')
                        for kc in range(nk):
                            _mm(nc, ps[:], stg[:, kc * 128:(kc + 1) * 128], rhs_fn(kc),
                                kc == 0, kc == nk - 1)
                        nc.vector.tensor_scalar_add(dst[:, mc * 8:(mc + 1) * 8], ps[:],
                                                    bias[:, mc:mc + 1])

                def cell_tail(rzt, hf):
                    """n = tanh(int_ + r*hnt); h = n + z*(h-n) in place."""
                    nc.vector.tensor_mul(ntt[:], rzt[:, 0:32], hnt[:])
                    nc.vector.tensor_add(ntt[:], ntt[:], int_[:])
                    nc.scalar.activation(nts[:], ntt[:], SIG, scale=2.0)
                    nc.vector.tensor_scalar(ntt[:], nts[:], 2.0, -1.0,
                                            mybir.AluOpType.mult, mybir.AluOpType.add)
                    nc.vector.tensor_sub(tta[:], hf[:], ntt[:])
                    nc.vector.tensor_mul(tta[:], rzt[:, 32:64], tta[:])
                    nc.vector.tensor_add(hf[:], ntt[:], tta[:])

                with tc.For_i(0, L) as i:
                    pre_t = prep.tile([128, 18 * 8], F32, tag='pre_t')
                    nc.sync.dma_start(pre_t[:], pre0d.ap()[ds(i * 128, 128)])

                    # q = wq @ h3 + bq
                    psQ = psA.tile([128, 8], F32, tag='gps')
                    for kc in range(4):
                        _mm(nc, psQ[:], wqT[:, kc * 128:(kc + 1) * 128],
                            h3f[:, kc * 8:(kc + 1) * 8], kc == 0, kc == 3)
                    nc.vector.tensor_scalar_add(qf[:], psQ[:], bq[:, 0:1])

                    # energy -> exp -> mask
                    for tcn in range(NT):
                        pse = psE.tile([128, 8], F32, tag='pse')
                        for b in range(BL):
                            col = (b * NT + tcn) * 128
                            _mm(nc, pse[:, b:b + 1], KT[:, col:col + 128],
                                qf[:, b:b + 1], True, True)
                        nc.scalar.activation(attb[:, tcn * 8:(tcn + 1) * 8], pse[:], EXP)
                        nc.vector.tensor_mul(attb[:, tcn * 8:(tcn + 1) * 8],
                                             attb[:, tcn * 8:(tcn + 1) * 8],
                                             maskp[:, tcn * 8:(tcn + 1) * 8])

                    # denom; broadcast 1/denom
                    psD = psA.tile([1, 8], F32, tag='gps')
                    for tcn in range(NT):
                        _mm(nc, psD[:], ones_c[:], attb[:, tcn * 8:(tcn + 1) * 8],
                            tcn == 0, tcn == NT - 1)
                    nc.vector.tensor_scalar_max(invd[:], psD[:], 1e-12)
                    nc.vector.reciprocal(invd[:], invd[:])
                    psB = psA.tile([128, 8], F32, tag='gps')
                    _mm(nc, psB[:], ones_r[:], invd[:], True, True)
                    nc.scalar.copy(invb[:], psB[:])

                    # ctx -> hv[:, 0:16] (fp32, scaled by 1/denom)
                    for vc in range(2):
                        psc = psC.tile([128, 8], F32, tag='psc')
                        for b in range(BL):
                            for tcn in range(NT):
                                col = ((b * NT + tcn) * 2 + vc) * 128
                                _mm(nc, psc[:, b:b + 1], VT[:, col:col + 128],
                                    attb[:, tcn * 8 + b:tcn * 8 + b + 1],
                                    tcn == 0, tcn == NT - 1)
                        nc.vector.tensor_mul(hv[:, vc * 8:(vc + 1) * 8], psc[:], invb[:])

                    # cell0
                    for mc in range(18):
                        ps = psA.tile([128, 8], F32, tag='gps')
                        for kc in range(2):
                            c = (kc * 18 + mc) * 128
                            _mm(nc, ps[:], W0[:, c:c + 128],
                                hv[:, kc * 8:(kc + 1) * 8], kc == 0, kc == 1)
                        if mc < 12:
                            nc.vector.tensor_add(rz0p[:, mc * 8:(mc + 1) * 8], ps[:],
                                                 pre_t[:, mc * 8:(mc + 1) * 8])
                        else:
                            j = mc - 12
                            nc.vector.tensor_scalar_add(hn0t[:, j * 8:(j + 1) * 8], ps[:],
                                                        hn0c[:, j:j + 1])
                    nc.scalar.activation(rz0[:], rz0p[:], SIG)
                    nc.vector.tensor_mul(t0[:], rz0[:, 0:48], hn0t[:])
                    nc.vector.tensor_add(t0[:], t0[:], pre_t[:, 96:144])
                    nc.scalar.activation(n0s[:], t0[:], SIG, scale=2.0)
                    nc.vector.tensor_scalar(n0[:], n0s[:], 2.0, -1.0,
                                            mybir.AluOpType.mult, mybir.AluOpType.add)
                    nc.vector.tensor_sub(t0[:], hv[:], n0[:])
                    nc.vector.tensor_mul(t0[:], rz0[:, 48:96], t0[:])
                    nc.vector.tensor_add(h0f[:], n0[:], t0[:])

                    # cell1: x = h0 (6 chunks), h = h1 (4 chunks)
                    def rhs1(kc):
                        return h0f[:, kc * 8:(kc + 1) * 8] if kc < 6 \
                            else h1f[:, (kc - 6) * 8:(kc - 5) * 8]
                    cell_fused(W1rz, 10, rhs1, brz1, rz1)
                    quad_stream(W1hnd, 4, lambda kc: h1f[:, kc * 8:(kc + 1) * 8],
                                bhn1, hnt)
                    quad_stream(W1ind, 6, lambda kc: h0f[:, kc * 8:(kc + 1) * 8],
                                bin1, int_)
                    cell_tail(rz1, h1f)

                    # cell2
                    def rhs2(kc):
                        return h1f[:, kc * 8:(kc + 1) * 8] if kc < 4 \
                            else h2f[:, (kc - 4) * 8:(kc - 3) * 8]
                    cell_fused_stream(W2rzd, 8, rhs2, brz2, rz2)
                    quad_stream(W2hnd, 4, lambda kc: h2f[:, kc * 8:(kc + 1) * 8],
                                bhn2, hnt)
                    quad_stream(W2ind, 4, lambda kc: h1f[:, kc * 8:(kc + 1) * 8],
                                bin2, int_)
                    cell_tail(rz2, h2f)

                    # cell3
                    def rhs3(kc):
                        return h2f[:, kc * 8:(kc + 1) * 8] if kc < 4 \
                            else h3f[:, (kc - 4) * 8:(kc - 3) * 8]
                    cell_fused_stream(W3rzd, 8, rhs3, brz3, rz3)
                    quad_stream(W3hnd, 4, lambda kc: h3f[:, kc * 8:(kc + 1) * 8],
                                bhn3, hnt)
                    quad_stream(W3ind, 4, lambda kc: h2f[:, kc * 8:(kc + 1) * 8],
                                bin3, int_)
                    cell_tail(rz3, h3f)

                    # spill h3_t to HBM history
                    for hc in range(4):
                        nc.sync.dma_start(hs[hc][:, ds(i * 8, 8)],
                                          h3f[:, hc * 8:(hc + 1) * 8])

                if DEBUG_STAGE:
                    for nm, tl in [('qf', qf), ('attb', attb), ('invb', invb),
                                   ('hv', hv), ('h0f', h0f), ('h1f', h1f),
                                   ('h2f', h2f)]:
                        nc.sync.dma_start(stgd[nm].ap(), tl[:])

            # ---------------- scoring ----------------
            with tc.tile_pool(name='sco', bufs=1) as sp, \
                 tc.tile_pool(name='scp', bufs=4, space='PSUM') as scps:
                h3s = [sp.tile([128, NL], F32, tag=f'h3s{k}', name=f'h3s{k}') for k in range(4)]
                for hc in range(4):
                    nc.sync.dma_start(h3s[hc][:], hs[hc][:])
                stats = sp.tile([128, 8], F32, tag='stats')
                statsg = sp.tile([128, 8], F32, tag='statsg')
                sq = sp.tile([128, HF], F32, tag='sq')
                tmp1 = sp.tile([128, 1], F32, tag='tmp1')
                mean4 = sp.tile([128, 4], F32, tag='mean4')
                var4 = sp.tile([128, 4], F32, tag='var4')
                sc1 = sp.tile([128, 4], F32, tag='sc1')
                sh1 = sp.tile([128, 4], F32, tag='sh1')
                sc2 = sp.tile([128, 4], F32, tag='sc2')
                sh2 = sp.tile([128, 4], F32, tag='sh2')
                b2f = sp.tile([128, 4], F32, tag='b2f')
                s1y = sp.tile([128, 4], F32, tag='s1y')
                s2y = sp.tile([128, 4], F32, tag='s2y')
                x1t = [sp.tile([128, HF], F32, tag=f'x1t{k}', name=f'x1t{k}') for k in range(4)]
                x2t = [sp.tile([128, HF], F32, tag=f'x2t{k}', name=f'x2t{k}') for k in range(4)]
                ybuf = sp.tile([128, 512], F32, tag='ybuf')
                scorest = sp.tile([34, NL], F32, tag='scorest')
                bnc1 = dp.tile([128, 8], F32, tag='bnc1')
                bnc1o = dp.tile([128, 8], F32, tag='bnc1o')
                bnc2 = dp.tile([128, 8], F32, tag='bnc2')
                bnc2o = dp.tile([128, 8], F32, tag='bnc2o')

                def bn_scale_shift(gt, bet, scX, shX):
                    nc.vector.tensor_scalar_mul(mean4[:], statsg[:, 0:4], 1.0 / NROWS)
                    nc.vector.tensor_scalar_mul(var4[:], statsg[:, 4:8], 1.0 / NROWS)
                    nc.vector.tensor_mul(sq[:, 0:4], mean4[:], mean4[:])
                    nc.vector.tensor_sub(var4[:], var4[:], sq[:, 0:4])
                    nc.vector.tensor_scalar_add(var4[:], var4[:], BN_EPS)
                    nc.scalar.activation(var4[:], var4[:], SQRT)
                    nc.vector.reciprocal(var4[:], var4[:])
                    nc.vector.tensor_mul(scX[:], gt[:], var4[:])
                    nc.vector.tensor_mul(shX[:], mean4[:], scX[:])
                    nc.vector.tensor_sub(shX[:], bet[:], shX[:])

                for hc in range(4):
                    nc.vector.reduce_sum(stats[:, hc:hc + 1], h3s[hc][:],
                                         axis=mybir.AxisListType.X)
                    for h in range(2):
                        nc.vector.tensor_mul(sq[:], h3s[hc][:, h * HF:(h + 1) * HF],
                                             h3s[hc][:, h * HF:(h + 1) * HF])
                        nc.vector.reduce_sum(tmp1[:], sq[:], axis=mybir.AxisListType.X)
                        if h == 0:
                            nc.vector.tensor_copy(stats[:, 4 + hc:5 + hc], tmp1[:])
                        else:
                            nc.vector.tensor_add(stats[:, 4 + hc:5 + hc],
                                                 stats[:, 4 + hc:5 + hc], tmp1[:])
                nc.gpsimd.dma_start(bnc1[:], stats[:])
                nc.gpsimd.collective_compute(
                    'AllReduce', mybir.AluOpType.add,
                    replica_groups=[list(range(N_CORES))],
                    ins=[bnc1.opt()], outs=[bnc1o.opt()])
                nc.gpsimd.dma_start(statsg[:], bnc1o[:])
                bn_scale_shift(g1, be1, sc1, sh1)

                nc.vector.memset(s1y[:], 0.0)
                nc.vector.memset(s2y[:], 0.0)
                for h in range(2):
                    for kc in range(4):
                        nc.scalar.activation(x1t[kc][:], h3s[kc][:, h * HF:(h + 1) * HF],
                                             SIG, bias=sh1[:, kc:kc + 1],
                                             scale=sc1[:, kc:kc + 1])
                    for mc in range(4):
                        for nb in range(2):
                            psy = scps.tile([128, 512], F32, tag='psy')
                            for kc in range(4):
                                _mm(nc, psy[:, 0:PS],
                                    w1T[:, (kc * 4 + mc) * 128:(kc * 4 + mc + 1) * 128],
                                    x1t[kc][:, nb * PS:(nb + 1) * PS], kc == 0, kc == 3)
                            nc.vector.tensor_scalar_add(ybuf[:, 0:PS], psy[:, 0:PS],
                                                        bb1[:, mc:mc + 1])
                            nc.vector.reduce_sum(tmp1[:], ybuf[:, 0:PS],
                                                 axis=mybir.AxisListType.X)
                            nc.vector.tensor_add(s1y[:, mc:mc + 1], s1y[:, mc:mc + 1],
                                                 tmp1[:])
                            nc.vector.tensor_mul(ybuf[:, 0:PS], ybuf[:, 0:PS],
                                                 ybuf[:, 0:PS])
                            nc.vector.reduce_sum(tmp1[:], ybuf[:, 0:PS],
                                                 axis=mybir.AxisListType.X)
                            nc.vector.tensor_add(s2y[:, mc:mc + 1], s2y[:, mc:mc + 1],
                                                 tmp1[:])
                nc.vector.tensor_copy(stats[:, 0:4], s1y[:])
                nc.vector.tensor_copy(stats[:, 4:8], s2y[:])
                nc.gpsimd.dma_start(bnc2[:], stats[:])
                nc.gpsimd.collective_compute(
                    'AllReduce', mybir.AluOpType.add,
                    replica_groups=[list(range(N_CORES))],
                    ins=[bnc2.opt()], outs=[bnc2o.opt()])
                nc.gpsimd.dma_start(statsg[:], bnc2o[:])
                bn_scale_shift(g2, be2, sc2, sh2)
                nc.vector.tensor_mul(b2f[:], bb1[:], sc2[:])
                nc.vector.tensor_add(b2f[:], b2f[:], sh2[:])

                for h in range(2):
                    for kc in range(4):
                        nc.scalar.activation(x1t[kc][:], h3s[kc][:, h * HF:(h + 1) * HF],
                                             SIG, bias=sh1[:, kc:kc + 1],
                                             scale=sc1[:, kc:kc + 1])
                    for mc in range(4):
                        for nb in range(2):
                            psy = scps.tile([128, 512], F32, tag='psy')
                            for kc in range(4):
                                _mm(nc, psy[:, 0:PS],
                                    w1T[:, (kc * 4 + mc) * 128:(kc * 4 + mc + 1) * 128],
                                    x1t[kc][:, nb * PS:(nb + 1) * PS], kc == 0, kc == 3)
                            nc.scalar.activation(x2t[mc][:, nb * PS:(nb + 1) * PS],
                                                 psy[:, 0:PS], SIG,
                                                 bias=b2f[:, mc:mc + 1],
                                                 scale=sc2[:, mc:mc + 1])
                    for nb in range(2):
                        pss = scps.tile([34, 512], F32, tag='pss')
                        for kc in range(4):
                            _mm(nc, pss[:, 0:PS], w2T[:, kc * 34:(kc + 1) * 34],
                                x2t[kc][:, nb * PS:(nb + 1) * PS], kc == 0, kc == 3)
                        nc.scalar.activation(
                            scorest[:, h * HF + nb * PS:h * HF + (nb + 1) * PS],
                            pss[:, 0:PS], IDN, bias=bb2[:, 0:1])
                nc.sync.dma_start(outd.ap(), scorest[:])
                if DEBUG_H3:
                    for hc in range(4):
                        nc.sync.dma_start(dbgd[hc].ap(), h3s[hc][:])

    nc.compile()
    return nc


def _chunks_kc(WT, nk, nm):
    """[nk*128, nm*128] -> [128, nk*nm*128], col (kc*nm+mc)*128+j (kc-major)."""
    return np.ascontiguousarray(
        WT.reshape(nk, 128, nm, 128).transpose(1, 0, 2, 3).reshape(128, -1))


def _chunks_mc(WT, nk, nm):
    """[nk*128, nm*128] -> [128, nm*nk*128], col (mc*nk+kc)*128+j (mc-major)."""
    return np.ascontiguousarray(
        WT.reshape(nk, 128, nm, 128).transpose(1, 2, 0, 3).reshape(128, -1))


def _pack_weights(params):
    p = {k: np.asarray(v, np.float32) if not isinstance(v, dict)
         else {kk: np.asarray(vv, np.float32) for kk, vv in v.items()}
         for k, v in params.items()}
    out = {}
    c0, c1, c2, c3 = p['cell0'], p['cell1'], p['cell2'], p['cell3']
    fh = p['first_hidden'][0]
    fhterm = c0['w_hh'][:, 256:] @ fh
    out['_pre0_base'] = (c0['w_ih'].T.copy(), c0['b_ih'], c0['b_hh'], fhterm)
    out['hn0p'] = (c0['b_hh'] + fhterm)[1536:].reshape(6, 128).T.copy()
    out['W0p'] = _chunks_kc(np.ascontiguousarray(c0['w_hh'][:, :256].T), 2, 18)

    def cell_pack(c, name, in_dim, stream_rz):
        nki = in_dim // 128
        rzcat = np.ascontiguousarray(
            np.concatenate([c['w_ih'][:1024].T, c['w_hh'][:1024].T], 0))
        out[f'W{name}rzp'] = (_chunks_mc if stream_rz else _chunks_kc)(rzcat, nki + 4, 8)
        out[f'W{name}inp'] = _chunks_mc(
            np.ascontiguousarray(c['w_ih'][1024:].T), nki, 4)
        hn = np.ascontiguousarray(c['w_hh'][1024:].T)
        out[f'W{name}hnp'] = _chunks_mc(hn, 4, 4)
        out[f'brz{name}p'] = (c['b_ih'] + c['b_hh'])[:1024].reshape(8, 128).T.copy()
        out[f'bin{name}p'] = c['b_ih'][1024:].reshape(4, 128).T.copy()
        out[f'bhn{name}p'] = c['b_hh'][1024:].reshape(4, 128).T.copy()

    cell_pack(c1, '1', 768, stream_rz=False)
    cell_pack(c2, '2', 512, stream_rz=True)
    cell_pack(c3, '3', 512, stream_rz=True)
    out['wqTp'] = _chunks_kc(np.ascontiguousarray(p['wq'].T), 4, 1)
    out['bqp'] = p['bq'].reshape(128, 1).copy()
    out['w1Tp'] = _chunks_kc(np.ascontiguousarray(p['w1'].T), 4, 4)
    out['w2Tp'] = np.ascontiguousarray(
        p['w2'].T.reshape(4, 128, 34).transpose(1, 0, 2).reshape(128, 136))
    for nm, key in [('g1p', 'g1'), ('be1p', 'be1'), ('bb1p', 'bb1'),
                    ('g2p', 'g2'), ('be2p', 'be2')]:
        out[nm] = p[key].reshape(4, 128).T.copy()
    out['bb2p'] = p['bb2'].reshape(34, 1).copy()
    for nm, key in [('h10p', 'init_h1'), ('h20p', 'init_h2'), ('h30p', 'init_h3')]:
        v = p[key][0].reshape(4, 128).T
        out[nm] = np.repeat(v[:, :, None], BL, axis=2).reshape(128, 32).copy()
    out['fhTp'] = np.repeat(fh.reshape(4, 128).T[:, :, None], BL,
                            axis=2).reshape(128, 32).copy()
    return out


def kernel(seq_list, keys, values, masks, params):
    global _cached_nc
    if _cached_nc is None:
        _cached_nc = _build()
    nc = _cached_nc

    seq_list = np.asarray(seq_list)
    keys = np.asarray(keys, np.float32)
    values = np.asarray(values, np.float32)
    masks = np.asarray(masks)
    shared = _pack_weights(params)
    wihT, bih0, bhh0, fhterm = shared.pop('_pre0_base')
    addrz = (bih0 + bhh0 + fhterm)[:1536]
    addn = bih0[1536:]

    in_maps = []
    for c in range(N_CORES):
        bs = slice(c * BL, (c + 1) * BL)
        m = dict(shared)
        kloc = keys[:, bs, :]
        m['KTp'] = np.ascontiguousarray(
            kloc.reshape(NT, 128, BL, 128).transpose(3, 2, 0, 1).reshape(128, -1))
        vloc = values[:, bs, :]
        m['VTp'] = np.ascontiguousarray(
            vloc.reshape(NT, 128, BL, 2, 128).transpose(1, 2, 0, 3, 4).reshape(128, -1))
        mk = (np.arange(T_ENC)[None, :] < masks[bs][:, None]).astype(np.float32)
        m['maskp'] = np.ascontiguousarray(
            mk.reshape(BL, NT, 128).transpose(2, 1, 0).reshape(128, NT * BL))
        pre = wihT[seq_list[bs]].transpose(1, 2, 0).copy()    # [250, 2304, 8]
        pre[:, :1536, :] += addrz[None, :, None]
        pre[:, 1536:, :] += addn[None, :, None]
        m['pre0p'] = np.ascontiguousarray(
            pre.reshape(L, 18, 128, BL).transpose(0, 2, 1, 3).reshape(L * 128, 18 * BL))
        in_maps.append(m)

    res = bass_utils.run_bass_kernel_spmd(nc, in_maps, core_ids=list(range(N_CORES)))
    out = np.empty((B, L, NUM_CHARS), np.float32)
    for c in range(N_CORES):
        sc = res.results[c]['outp']                    # [34, 2000], col t*8+b
        out[c * BL:(c + 1) * BL] = sc.reshape(34, L, BL).transpose(2, 1, 0)
    if DEBUG_H3:
        h3 = np.empty((B, L, HID), np.float32)
        for c in range(N_CORES):
            for hc in range(4):
                d = res.results[c][f'dbg{hc}']          # [128, L*BL]
                h3[c * BL:(c + 1) * BL, :, hc * 128:(hc + 1) * 128] =                     d.reshape(128, L, BL).transpose(2, 1, 0)
        return out, h3
    return out
